# revision 1
# baseline (speedup 1.0000x reference)
"""Trainium2 Bass kernel for nn_Decoder_35527969472565 (v2).

Contract: kernel(**inputs) takes the FULL unsharded inputs (as produced by
setup_inputs()) and returns the FULL [32, 400, 80] float32 output.

Design notes
- The attention softmax is invariant to the per-(b,t) additive score term, so
  the attention context is step-independent: computed once, folded (with b0
  and the x_t @ Wih_x term) into a per-step "aug" matmul whose lhsT is the
  host-built [x_t; 0; I32] stack.
- Recurrence gates live in 2 PSUM banks of [128, 512] each: partition
  p = 32*j + b holds batch row b of gate group 4*s+j (s = bank/stack,
  j = PE column-tile). The four groups of a bank are computed by 4-way
  col-tiled matmuls (tile_position=(0,32j)) that run concurrently on the PE
  array, with bf16 operands (1 cycle/row vs fp32's 4).
- h is transposed back to lhsT layout with one PE transpose per stack.
- All big GEMM streams (Whh0/Wih1/Whh1/Wfc, Q1/H0/H1 scratch) are bf16;
  PSUM accumulation and the c state stay fp32.
- Single-core program replicated across the 8 NeuronCores (the serial
  recurrence dominates; batch sharding does not reduce the weight-streaming
  cost that bounds each step).
"""

import numpy as np
import ml_dtypes

import concourse.bacc as bacc
import concourse.mybir as mybir
import concourse.tile as tile

F32 = mybir.dt.float32
BF16 = mybir.dt.bfloat16
AF = mybir.ActivationFunctionType
BF = ml_dtypes.bfloat16

B = 32
S = 200
E2 = 512
H = 1024
M = 80
G4 = 4 * H      # 4096
NG = 8          # gate groups of 512 cols each
GW = 512


def gate_perm():
    P = []
    for n in range(NG):
        P += list(range(0 * H + n * 128, 0 * H + (n + 1) * 128))   # i
        P += list(range(1 * H + n * 128, 1 * H + (n + 1) * 128))   # f
        P += list(range(3 * H + n * 128, 3 * H + (n + 1) * 128))   # o
        P += list(range(2 * H + n * 128, 2 * H + (n + 1) * 128))   # g
    return np.array(P)


def _rearr(wT, c):
    # [K, N] with K = c*128 -> [128, c*N] laid out as p, (c n)
    K, N = wT.shape
    assert K == c * 128
    return np.ascontiguousarray(
        wT.reshape(c, 128, N).transpose(1, 0, 2).reshape(128, c * N))


def prep_inputs(inp, T):
    P = gate_perm()
    f32 = np.float32
    Wih0 = np.asarray(inp["Wih0"], f32)[P]
    Whh0 = np.asarray(inp["Whh0"], f32)[P]
    Wih1 = np.asarray(inp["Wih1"], f32)[P]
    Whh1 = np.asarray(inp["Whh1"], f32)[P]
    b0 = (np.asarray(inp["bih0"], f32) + np.asarray(inp["bhh0"], f32))[P]
    b1 = (np.asarray(inp["bih1"], f32) + np.asarray(inp["bhh1"], f32))[P]
    enc = np.asarray(inp["encoder_outputs"], f32)
    x = np.asarray(inp["audio_targets"], f32)[:, :T, :]

    d = {}
    # attention (f32)
    d["enc"] = np.ascontiguousarray(enc).astype(BF)
    d["waeRep"] = np.tile(np.asarray(inp["Wa"], f32)[0:1, H:], (128, 1))
    d["w0col"] = np.full((S, 1), 1.0 / S, f32).astype(BF)
    # C0D build (bf16)
    d["wihe4"] = _rearr(np.ascontiguousarray(Wih0[:, M:].T), 4).astype(BF)
    d["b0row"] = b0.reshape(1, G4).astype(BF)
    # aug input stack: rows 0:80 = x_t, 80:96 = 0, 96:128 = I32 (per step)
    xT = x.transpose(2, 1, 0).reshape(M, T * B)        # [80, T*B]
    xAug = np.zeros((128, T * B), f32)
    xAug[:M] = xT
    eye = np.eye(B, dtype=f32)
    xAug[96:128] = np.tile(eye, (1, T))
    d["xAugT"] = xAug.astype(BF)
    wxp = np.zeros((96, G4), f32)
    wxp[:M] = Wih0[:, :M].T
    d["WxPad"] = wxp.astype(BF)
    # big weight streams
    d["WhhT0r"] = _rearr(np.ascontiguousarray(Whh0.T), 8).astype(BF)
    d["WihT1r"] = _rearr(np.ascontiguousarray(Wih1.T), 8).astype(BF)
    d["WhhT1r"] = _rearr(np.ascontiguousarray(Whh1.T), 8).astype(BF)
    d["b1row"] = b1.reshape(1, G4).astype(BF)
    d["WfcTr"] = _rearr(np.ascontiguousarray(np.asarray(inp["Wfc"], f32).T),
                        8).astype(BF)
    d["bfcrow"] = np.asarray(inp["bfc"], f32).reshape(1, M).astype(BF)
    i33 = np.zeros((B + 1, B), f32)
    i33[:B] = np.eye(B)
    i33[B] = 1.0
    d["I33bf"] = i33.astype(BF)
    d["ones128"] = np.ones((1, 128), f32).astype(BF)
    d["identbf"] = np.eye(128, dtype=f32).astype(BF)
    return d


SHAPES = {
    "enc": ((B, S, E2), BF16),
    "waeRep": ((128, E2), F32),
    "w0col": ((S, 1), BF16),
    "wihe4": ((128, 4 * G4), BF16),
    "b0row": ((1, G4), BF16),
    "WxPad": ((96, G4), BF16),
    "WhhT0r": ((128, 8 * G4), BF16),
    "WihT1r": ((128, 8 * G4), BF16),
    "WhhT1r": ((128, 8 * G4), BF16),
    "b1row": ((1, G4), BF16),
    "WfcTr": ((128, 8 * M), BF16),
    "bfcrow": ((1, M), BF16),
    "I33bf": ((B + 1, B), BF16),
    "ones128": ((1, 128), BF16),
    "identbf": ((128, 128), BF16),
}


def build(T=400):
    nc = bacc.Bacc()
    t_in = {}
    for n, (shape, dt) in SHAPES.items():
        t_in[n] = nc.dram_tensor(n, list(shape), dt, kind="ExternalInput")
    xAugT = nc.dram_tensor("xAugT", [128, T * B], BF16, kind="ExternalInput")
    out = nc.dram_tensor("out", [B, T, M], F32, kind="ExternalOutput")

    C0D = nc.dram_tensor("C0D", [2, B, G4], BF16)   # [0]=run, [1]=init
    H0T = nc.dram_tensor("H0T", [T, 128, 256], BF16)
    H1T = nc.dram_tensor("H1T", [T, 128, 256], BF16)
    Q1 = nc.dram_tensor("Q1", [T * B, G4], BF16)

    with tile.TileContext(nc) as tc, \
         tc.tile_pool(name="wp", bufs=1) as wp, \
         tc.tile_pool(name="sb", bufs=2) as sb, \
         tc.tile_pool(name="sb3", bufs=3) as sb3:
        # ---------- resident small tensors ----------
        identt = wp.tile([128, 128], BF16, tag="identt")
        nc.sync.dma_start(identt[:], t_in["identbf"][:])
        i33t = wp.tile([B + 1, B], BF16, tag="i33t")
        nc.sync.dma_start(i33t[:], t_in["I33bf"][:])
        ones128t = wp.tile([1, 128], BF16, tag="ones128t")
        nc.sync.dma_start(ones128t[:], t_in["ones128"][:])
        waer = wp.tile([128, E2], F32, tag="waer")
        nc.sync.dma_start(waer[:], t_in["waeRep"][:])
        w0a = wp.tile([128, 1], BF16, tag="w0a")
        nc.sync.dma_start(w0a[:], t_in["w0col"][0:128, :])
        w0c = wp.tile([72, 1], BF16, tag="w0c")
        nc.sync.dma_start(w0c[:], t_in["w0col"][128:200, :])
        b0r = wp.tile([1, G4], BF16, tag="b0r")
        nc.sync.dma_start(b0r[:], t_in["b0row"][:])
        bfcr = wp.tile([1, M], BF16, tag="bfcr")
        nc.sync.dma_start(bfcr[:], t_in["bfcrow"][:])

        with tc.tile_pool(name="encp", bufs=1) as encp, \
             tc.tile_pool(name="sba", bufs=2) as sba, \
             tc.tile_pool(name="psa", bufs=1, space="PSUM") as psa:
            # enc resident in SBUF for the whole attention section
            encA = encp.tile([128, B * E2], BF16, tag="encA")
            encC = encp.tile([72, B * E2], BF16, tag="encC")
            for b in range(B):
                nc.sync.dma_start(encA[:, E2 * b:E2 * (b + 1)],
                                  t_in["enc"][b, 0:128, :])
                nc.sync.dma_start(encC[:, E2 * b:E2 * (b + 1)],
                                  t_in["enc"][b, 128:200, :])

            # ---------- context: ctxT tiles (bf16 out) ----------
            def ctx_tiles(rcol_a, rcol_c, tagbase):
                cps = [psa.tile([128, B], F32, tag=f"b{hs}",
                                name=f"ctxps_{tagbase}_{hs}")
                       for hs in range(4)]
                for b in range(B):
                    ea = encA[:, E2 * b:E2 * (b + 1)]
                    ec = encC[:, E2 * b:E2 * (b + 1)]
                    for hs in range(4):
                        nc.tensor.matmul(cps[hs][:, b:b + 1],
                                         ea[:, 128 * hs:128 * (hs + 1)],
                                         rcol_a(b), start=True, stop=False)
                        nc.tensor.matmul(cps[hs][:, b:b + 1],
                                         ec[:, 128 * hs:128 * (hs + 1)],
                                         rcol_c(b), start=False, stop=True)
                outs = []
                for hs in range(4):
                    ct = wp.tile([128, B], BF16, tag=f"{tagbase}{hs}")
                    nc.scalar.activation(ct[:], cps[hs][:], AF.Copy)
                    outs.append(ct)
                return outs

            # ctx0 (mean context) is independent of the scores: run its PE
            # passes under the DVE/ACT score computation
            ctx0T = ctx_tiles(lambda b: w0a[:], lambda b: w0c[:], "c0T")

            # ---------- attention: scores via DVE reduce (f32) ----------
            scT_a = wp.tile([128, B], F32, tag="scT_a")
            scT_c = wp.tile([96, B], F32, tag="scT_c")
            nc.vector.memset(scT_c[:], 0.0)
            for b in range(B):
                ea = encA[:, E2 * b:E2 * (b + 1)]
                ec = encC[:, E2 * b:E2 * (b + 1)]
                tha = sba.tile([128, E2], F32, tag="tha", name=f"tha{b}")
                nc.scalar.activation(tha[:], ea, AF.Tanh)
                thc = sba.tile([72, E2], F32, tag="thc")
                nc.scalar.activation(thc[:], ec, AF.Tanh)
                pra = sba.tile([128, E2], F32, tag="pra", name=f"pra{b}")
                nc.vector.tensor_mul(out=pra[:], in0=tha[:], in1=waer[:])
                prc = sba.tile([72, E2], F32, tag="prc", name=f"prc{b}")
                nc.vector.tensor_mul(out=prc[:], in0=thc[:], in1=waer[0:72, :])
                nc.vector.reduce_sum(scT_a[:, b:b + 1], pra[:],
                                     axis=mybir.AxisListType.X)
                nc.vector.reduce_sum(scT_c[0:72, b:b + 1], prc[:],
                                     axis=mybir.AxisListType.X)
            score = wp.tile([B, 224], F32, tag="score")
            for j in range(4):
                nc.vector.transpose(score[:, 32 * j:32 * (j + 1)],
                                    scT_a[32 * j:32 * (j + 1), :])
            for j in range(3):
                nc.vector.transpose(score[:, 128 + 32 * j:160 + 32 * j],
                                    scT_c[32 * j:32 * (j + 1), :])

            mx = sb.tile([B, 1], F32, tag="mx")
            nc.vector.reduce_max(mx[:], score[:, 0:S],
                                 axis=mybir.AxisListType.X)
            nmx = sb.tile([B, 1], F32, tag="nmx")
            nc.vector.tensor_scalar_mul(nmx[:], mx[:], -1.0)
            ew = wp.tile([B, 224], F32, tag="ew")
            nc.vector.memset(ew[:], 0.0)
            nc.scalar.activation(ew[:, 0:S], score[:, 0:S], AF.Exp,
                                 bias=nmx[:])
            sm = sb.tile([B, 1], F32, tag="sm")
            nc.vector.reduce_sum(sm[:], ew[:, 0:S], axis=mybir.AxisListType.X)
            rs = sb.tile([B, 1], F32, tag="rs")
            nc.vector.reciprocal(rs[:], sm[:])
            wgt = wp.tile([B, 224], F32, tag="wgt")
            nc.vector.tensor_scalar_mul(wgt[:], ew[:], rs[:])
            wT_a = wp.tile([128, B], F32, tag="wT_a")
            wT_c = wp.tile([96, B], F32, tag="wT_c")
            for j in range(4):
                nc.vector.transpose(wT_a[32 * j:32 * (j + 1), :],
                                    wgt[:, 32 * j:32 * (j + 1)])
            for j in range(3):
                nc.vector.transpose(wT_c[32 * j:32 * (j + 1), :],
                                    wgt[:, 128 + 32 * j:128 + 32 * (j + 1)])
            wTab = wp.tile([128, B], BF16, tag="wTab")
            nc.vector.tensor_copy(wTab[:], wT_a[:])
            wTcb = wp.tile([96, B], BF16, tag="wTcb")
            nc.vector.tensor_copy(wTcb[:], wT_c[:])

            ctxT = ctx_tiles(lambda b: wTab[:, b:b + 1],
                             lambda b: wTcb[0:72, b:b + 1], "cT")

            # ---------- C0D = ctx @ Wihe + b0  (bf16) ----------
            wihe = encp.tile([128, 4 * G4], BF16, tag="wihe")
            nc.sync.dma_start(wihe[:], t_in["wihe4"][:])
            for idx, ctx_t in ((0, ctxT), (1, ctx0T)):
                for n in range(NG):
                    cps = psa.tile([B, GW], F32, tag="cg",
                                   name=f"c0ps_{idx}_{n}")
                    nc.tensor.matmul(cps[:], ones128t[:, 0:B],
                                     b0r[:, GW * n:GW * (n + 1)],
                                     start=True, stop=False)
                    for hs in range(4):
                        nc.tensor.matmul(
                            cps[:], ctx_t[hs][:],
                            wihe[:, G4 * hs + GW * n:G4 * hs + GW * (n + 1)],
                            start=False, stop=(hs == 3))
                    csb = sb3.tile([B, GW], BF16, tag="csb")
                    nc.scalar.activation(csb[:], cps[:], AF.Copy)
                    nc.sync.dma_start(C0D[idx, :, GW * n:GW * (n + 1)], csb[:])

        # ---------- aug rhs tiles: [WxPad; C0D[v]] ----------
        wxa = []
        for v in range(2):
            w = wp.tile([128, G4], BF16, tag=f"wxa{v}")
            nc.sync.dma_start(w[0:96, :], t_in["WxPad"][:])
            nc.sync.dma_start(w[96:128, :], C0D[v])
            wxa.append(w)

        with tc.tile_pool(name="wb", bufs=1) as wb, \
             tc.tile_pool(name="psg", bufs=2, space="PSUM") as psg, \
             tc.tile_pool(name="pst", bufs=1, space="PSUM") as pst, \
             tc.tile_pool(name="psm", bufs=2, space="PSUM") as psm:

            # ---------- shared recurrence step ----------
            # extra(t) is called after each step's emission: it injects
            # independent matmul work (mid GEMM / fc) into the PE stream to
            # fill the gaps where the PE would otherwise wait on the
            # elementwise chain of the recurrence.
            def recur_phase(big, inject, HT, extra_mm=None,
                            extra_post=None):
                c_prev = None
                hT_prev = None
                for t in range(T):
                    Gs = [psg.tile([128, GW], F32, tag=f"g{s}",
                                   name=f"G{s}_{t}") for s in range(2)]
                    inject(t, Gs)
                    if hT_prev is not None:
                        for s in range(2):
                            for c in range(8):
                                for j in range(4):
                                    nc.tensor.matmul(
                                        Gs[s][32 * j:32 * (j + 1), :],
                                        hT_prev[:, 32 * c:32 * (c + 1)],
                                        big[:, G4 * c + GW * (4 * s + j):
                                            G4 * c + GW * (4 * s + j + 1)],
                                        start=False, stop=(c == 7),
                                        skip_group_check=True,
                                        tile_position=(0, 32 * j))
                    if extra_mm is not None:
                        extra_mm(t)
                    c_new = sb.tile([128, 256], F32, tag="c", name=f"c_{t}")
                    hT_new = sb.tile([128, 256], BF16, tag="hT",
                                     name=f"hT_{t}")
                    hs_t = []
                    sig = []
                    tg = []
                    for s in range(2):
                        sg = sb.tile([128, 384], BF16, tag=f"sig{s}")
                        nc.scalar.activation(sg[:], Gs[s][:, 0:384],
                                             AF.Sigmoid)
                        sig.append(sg)
                        tgs = sb.tile([128, 128], BF16, tag=f"tg{s}")
                        nc.scalar.activation(tgs[:], Gs[s][:, 384:512],
                                             AF.Tanh)
                        tg.append(tgs)
                    for s in range(2):
                        csl = c_new[:, 128 * s:128 * (s + 1)]
                        t2 = sb.tile([128, 128], BF16, tag=f"t2{s}")
                        nc.vector.tensor_mul(out=t2[:], in0=sig[s][:, 0:128],
                                             in1=tg[s][:])
                        if c_prev is None:
                            nc.vector.tensor_copy(csl, t2[:])
                        else:
                            t1 = sb.tile([128, 128], F32, tag=f"t1{s}")
                            nc.vector.tensor_mul(
                                out=t1[:], in0=sig[s][:, 128:256],
                                in1=c_prev[:, 128 * s:128 * (s + 1)])
                            nc.vector.tensor_add(out=csl, in0=t1[:],
                                                 in1=t2[:])
                        tcs = sb.tile([128, 128], BF16, tag=f"tc{s}")
                        nc.scalar.activation(tcs[:], csl, AF.Tanh)
                        hsl = sb.tile([128, 128], BF16, tag=f"h{s}")
                        nc.vector.tensor_mul(out=hsl[:],
                                             in0=sig[s][:, 256:384],
                                             in1=tcs[:])
                        hs_t.append(hsl)
                    for s in range(2):
                        tp = pst.tile([128, 128], BF16, tag=f"tp{s}")
                        nc.tensor.transpose(tp[:], hs_t[s][:], identt[:])
                        nc.vector.tensor_copy(
                            hT_new[:, 128 * s:128 * (s + 1)], tp[:])
                    nc.scalar.dma_start(HT[t], hT_new[:])
                    c_prev, hT_prev = c_new, hT_new
                    if extra_post is not None:
                        extra_post(t)

            assert T % 4 == 0
            NM = T // 4

            # ---------- phase A: layer-0 recurrence + fused mid GEMM ----------
            big = wb.tile([128, 8 * G4], BF16, tag="bigW")
            nc.sync.dma_start(big[:], t_in["WhhT0r"][:])
            big2 = wb.tile([128, 8 * G4], BF16, tag="bigW2")
            nc.sync.dma_start(big2[:], t_in["WihT1r"][:])

            def inject_A(t, Gs):
                xa = sb3.tile([128, B], BF16, tag="xa")
                nc.sync.dma_start(xa[:], xAugT[:, B * t:B * (t + 1)])
                w = wxa[1 if t == 0 else 0]
                for s in range(2):
                    for j in range(4):
                        nc.tensor.matmul(
                            Gs[s][32 * j:32 * (j + 1), :], xa[:],
                            w[:, GW * (4 * s + j):GW * (4 * s + j + 1)],
                            start=True, stop=(t == 0),
                            skip_group_check=True, tile_position=(0, 32 * j))

            # mid GEMM Q1 = H0 @ Wih1T, emitted as (m, n) column-group
            # passes interleaved into phase A's PE stream
            mid = {"i": 0, "dma_m": -1, "slabs": {}, "pend": []}
            NPAIR = NM * NG
            LAG = 4   # consume slabs this many steps after availability so
                      # the next slab's DMA can be issued with real lead time

            def mid_ensure(m, t):
                # DMA slabs up to m+1 (prefetch) as soon as their H0T rows
                # are written
                target = min(NM - 1, m + 1)
                while (mid["dma_m"] < target
                       and 4 * (mid["dma_m"] + 1) + 3 < t):
                    mm = mid["dma_m"] + 1
                    slab = sb3.tile([128, 1024], BF16, tag="slab3",
                                    name=f"mslab{mm}")
                    nc.sync.dma_start(
                        slab[:],
                        H0T.ap()[4 * mm:4 * (mm + 1)].rearrange(
                            "t p (c b) -> p c t b", c=8))
                    mid["slabs"][mm] = slab
                    mid["slabs"].pop(mm - 3, None)
                    mid["dma_m"] = mm

            def mid_mm(t, budget, cap=None):
                limit = NPAIR if cap is None else cap
                k = 0
                while k < budget and mid["i"] < limit:
                    m, n = divmod(mid["i"], NG)
                    if 4 * m + 3 + LAG > t:
                        break
                    mid_ensure(m, t)
                    slab = mid["slabs"][m]
                    qps = psm.tile([128, GW], F32, tag="gm",
                                   name=f"q_{m}_{n}")
                    for c in range(8):
                        nc.tensor.matmul(
                            qps[:], slab[:, 128 * c:128 * (c + 1)],
                            big2[:, G4 * c + GW * n:G4 * c + GW * (n + 1)],
                            start=(c == 0), stop=(c == 7))
                    mid["pend"].append((qps, m, n))
                    mid["i"] += 1
                    k += 1

            def mid_post(t):
                pend, mid["pend"] = mid["pend"], []
                for idx, (qps, m, n) in enumerate(pend):
                    qsb = sb.tile([128, GW], BF16, tag="qsb",
                                  name=f"qsb_{m}_{n}")
                    if idx % 2 == 0:
                        nc.scalar.activation(qsb[:], qps[:], AF.Copy)
                    else:
                        nc.vector.tensor_copy(qsb[:], qps[:])
                    nc.sync.dma_start(
                        Q1[128 * m:128 * (m + 1), GW * n:GW * (n + 1)],
                        qsb[:])

            # leave the last mid passes for phase B's slack; producing 1/step
            # from B step 0 stays far ahead of the inject consuming those
            # late Q1 rows (deferral bounded so production always leads)
            A_CAP = NPAIR - min(96, max(0, 4 * NM - 12))
            recur_phase(big, inject_A, H0T,
                        extra_mm=lambda t: mid_mm(t, 2, A_CAP),
                        extra_post=lambda t: mid_post(t))
            while mid["i"] < A_CAP:   # drain A's share
                mid_mm(10 ** 9, 2, A_CAP)
                mid_post(10 ** 9)

            # ---------- phase B: layer-1 recurrence + fused fc ----------
            big = wb.tile([128, 8 * G4], BF16, tag="bigW")
            nc.sync.dma_start(big[:], t_in["WhhT1r"][:])
            wfc = wb.tile([128, 8 * M], BF16, tag="wfc")
            nc.sync.dma_start(wfc[:], t_in["WfcTr"][:])

            def inject_B(t, Gs):
                # rows 0:32 = Q1[t] (no bias), row 32 = b1 -> lhsT I33 adds it
                q1t = q1p.tile([B + 1, G4], BF16, tag="q1")
                nc.sync.dma_start(q1t[0:B, :], Q1[B * t:B * (t + 1), :])
                nc.sync.dma_start(q1t[B:B + 1, :], t_in["b1row"][:])
                for s in range(2):
                    for j in range(4):
                        nc.tensor.matmul(
                            Gs[s][32 * j:32 * (j + 1), :], i33t[:],
                            q1t[:, GW * (4 * s + j):GW * (4 * s + j + 1)],
                            start=True, stop=(t == 0),
                            skip_group_check=True, tile_position=(0, 32 * j))

            fc = {"m": 0, "pend": []}

            fc["dma_m"] = -1
            fc["slabs"] = {}

            def fc_ensure(m, t):
                target = min(NM - 1, m + 1)
                while (fc["dma_m"] < target
                       and 4 * (fc["dma_m"] + 1) + 3 < t):
                    mm = fc["dma_m"] + 1
                    slab = sb3.tile([128, 1024], BF16, tag="slab3",
                                    name=f"fslab{mm}")
                    nc.sync.dma_start(
                        slab[:],
                        H1T.ap()[4 * mm:4 * (mm + 1)].rearrange(
                            "t p (c b) -> p c t b", c=8))
                    fc["slabs"][mm] = slab
                    fc["slabs"].pop(mm - 3, None)
                    fc["dma_m"] = mm

            def fc_mm(t, budget):
                k = 0
                while (k < budget and fc["m"] < NM
                       and 4 * fc["m"] + 3 + 4 <= t):
                    m = fc["m"]
                    fc_ensure(m, t)
                    slab = fc["slabs"][m]
                    pfull = psm.tile([128, GW], F32, tag="gm",
                                     name=f"p_{m}")
                    pps = pfull[:, 0:M]
                    nc.tensor.matmul(pps, ones128t[:], bfcr[:],
                                     start=True, stop=False)
                    for c in range(8):
                        nc.tensor.matmul(pps,
                                         slab[:, 128 * c:128 * (c + 1)],
                                         wfc[:, M * c:M * (c + 1)],
                                         start=False, stop=(c == 7))
                    fc["pend"].append((pfull, m))
                    fc["m"] += 1
                    k += 1

            def fc_post(t):
                pend, fc["pend"] = fc["pend"], []
                for pfull, m in pend:
                    pout = sb.tile([128, M], F32, tag="qsb2",
                                   name=f"po{m}")
                    nc.scalar.activation(pout[:], pfull[:, 0:M], AF.Copy)
                    for tt in range(4):
                        nc.sync.dma_start(out[:, 4 * m + tt, :],
                                          pout[32 * tt:32 * (tt + 1), :])

            def extra_B_mm(t):
                fc_mm(t, 1)
                mid_mm(10 ** 9, 1)   # tail of the mid GEMM in B's slack

            def extra_B_post(t):
                fc_post(t)
                mid_post(t)

            with tc.tile_pool(name="q1p", bufs=2) as q1p:
                recur_phase(big, inject_B, H1T,
                            extra_mm=extra_B_mm,
                            extra_post=extra_B_post)
            while fc["m"] < NM or fc["pend"] or mid["i"] < NPAIR:   # drain
                fc_mm(10 ** 9, 1)
                mid_mm(10 ** 9, 2)
                fc_post(10 ** 9)
                mid_post(10 ** 9)
    nc.finalize()
    return nc, list(SHAPES) + ["xAugT"]


_CACHE = {}


def kernel(**inputs):
    import numpy as np
    from concourse.bass_utils import run_bass_kernel_spmd

    T = int(np.asarray(inputs["audio_targets"]).shape[1])
    if T not in _CACHE:
        _CACHE[T] = build(T)
    nc, _names = _CACHE[T]
    d = prep_inputs(inputs, T)
    n_cores = 8
    in_maps = [dict(d) for _ in range(n_cores)]
    res = run_bass_kernel_spmd(nc, in_maps, list(range(n_cores)))
    return np.asarray(res.results[0]["out"], dtype=np.float32)

